# revision 1
# baseline (speedup 1.0000x reference)
"""Trainium2 Bass kernel for CausalSelfAttention with kron-structured bias and
column-masked causal attention.

Shapes (hardcoded): x (4,1024,512), H=8 heads, HD=64, attn_bias (8,64,64)
expanded by kron(ones(8,8)) onto the top-left 512x512 of the (1024,1024)
score matrix. Causal tril mask with every 16th column (j%16==15) zeroed.

Sharding: 8 cores = 4 batches x 2 head-groups (4 heads each). Every core runs
an identical program on its own slice:
  - computes Q^T,K^T (feature-major) and V (token-major, with fused bias and a
    ones column per head for the softmax denominator) for its 4 heads,
  - scores S^T = K^T.T @ Q^T per (head, 128-row key tile) with the kron bias
    folded in as a rank-64 second matmul (E^T, repeat(attn_bias)) accumulating
    into the same PSUM tile,
  - P^T = exp(S^T + colmask_bias) on ScalarE (column mask via per-partition
    bias of -1e30), triangular mask on the single diagonal-crossing 128-col
    block via a 0/1 multiply,
  - O^T_ext = V_ext.T @ P^T accumulated over key tiles (row 64 = softmax
    denominator), normalized via reciprocal + ones-broadcast matmul,
  - partial projection Z = y^T.T @ Wp^T slice. Host sums the two per-batch
    partials and adds bp.

All matmuls run as float32r (4-byte fp32 data, fast PE streaming mode).
"""

import os
import sys
import types

import numpy as np

import concourse.bass as bass
import concourse.bacc as bacc
import concourse.tile as tile
from concourse import mybir
from concourse.bass_utils import run_bass_kernel_spmd


def _ensure_axon_hooks():
    """bass_utils' trace path imports antenv.axon_hooks unconditionally; some
    images lack that module. Provide it (and register the real NTFF hook when
    the axon boot shim is available) so tracing degrades gracefully."""
    try:
        import antenv.axon_hooks  # noqa: F401
        return
    except ImportError:
        pass
    m = types.ModuleType("antenv.axon_hooks")
    m._hook = None
    m.set_axon_ntff_profile_hook = lambda h: setattr(m, "_hook", h)
    m.get_axon_ntff_profile_hook = lambda: m._hook
    sys.modules["antenv.axon_hooks"] = m
    try:
        import antenv
        antenv.axon_hooks = m
    except ImportError:
        pass
    try:
        from trn_agent_boot.trn_boot import _ntff_profile_via_ctypes
        m.set_axon_ntff_profile_hook(
            _ntff_profile_via_ctypes("/opt/axon/libaxon_pjrt.so")
        )
    except Exception:
        pass


_ensure_axon_hooks()

F32 = mybir.dt.float32
F32R = mybir.dt.float32r
AFT = mybir.ActivationFunctionType

B, T, C, H = 4, 1024, 512, 8
HD = 64
SCALE = 1.0 / 8.0
GH = 4          # heads per core
N_CORES = 8

_CACHE = {}
LAST_RESULTS = None


def _kernel_body(tc, io, stage=99):
    nc = tc.nc
    xT, WqT, WkT, WvE, WpT, BQ, BK, BMQ, ET, TRI, CMASK, Z = (
        io["xT"], io["WqT"], io["WkT"], io["WvE"], io["WpT"], io["BQ"],
        io["BK"], io["BMQ"], io["ET"], io["TRI"], io["CMASK"], io["Z"],
    )
    ONE8J, ZERO7 = io["ONE8J"], io["ZERO7"]

    from contextlib import ExitStack
    with ExitStack() as ctx:
        const = ctx.enter_context(tc.tile_pool(name="const", bufs=1))
        pmm = ctx.enter_context(tc.tile_pool(name="pmm", bufs=2, space="PSUM"))
        ps = ctx.enter_context(tc.tile_pool(name="ps", bufs=2, space="PSUM"))
        pot = ctx.enter_context(tc.tile_pool(name="pot", bufs=3, space="PSUM"))
        pbc = ctx.enter_context(tc.tile_pool(name="pbc", bufs=1, space="PSUM"))
        spt = ctx.enter_context(tc.tile_pool(name="spt", bufs=4))
        sden = ctx.enter_context(tc.tile_pool(name="sden", bufs=3))
        szout = ctx.enter_context(tc.tile_pool(name="szout", bufs=2))

        # ---- persistent SBUF tiles (unique tags in bufs=1 pool)
        def ctile(shape, tag, dt=F32R):
            return const.tile(shape, dt, tag=tag, name=tag)

        xt = [ctile([128, T], f"xt{i}") for i in range(4)]
        wq = [ctile([128, 256], f"wq{i}") for i in range(4)]
        wk = [ctile([128, 256], f"wk{i}") for i in range(4)]
        wv = [ctile([128, 260], f"wv{i}") for i in range(4)]
        wv5 = ctile([8, 260], "wv5")
        wp = [ctile([128, 512], f"wp{i}") for i in range(2)]
        bmq = [ctile([128, T], f"bmq{i}") for i in range(GH)]
        et = ctile([128, T], "et")
        tri = ctile([128, 128], "tri")
        cmask = ctile([128, 1], "cmask", F32)
        bq_t = ctile([128, 2], "bq_t", F32)
        bk_t = ctile([128, 2], "bk_t", F32)
        one8j = ctile([8, 128], "one8j")
        one8b = ctile([72, 128], "one8b")
        qt = [ctile([128, T], f"qt{i}") for i in range(2)]
        kt = [ctile([128, T], f"kt{i}") for i in range(2)]
        ve = [ctile([128, 260], f"ve{i}") for i in range(8)]
        yt = [ctile([128, T], f"yt{i}") for i in range(2)]

        # ---- loads: partition-chunked so each big tile spreads over several
        # HWDGE queues, critical-path tiles (x, Wq, Wk) first
        def chunked(dst, src, n):
            p = dst.shape[0] // n
            for c in range(n):
                nc.sync.dma_start(
                    out=dst[c * p:(c + 1) * p, :], in_=src[c * p:(c + 1) * p, :]
                )

        for i in range(4):
            chunked(xt[i], xT[i * 128:(i + 1) * 128, :], 4)
            chunked(wq[i], WqT[i * 128:(i + 1) * 128, :], 2)
            chunked(wk[i], WkT[i * 128:(i + 1) * 128, :], 2)
        for i in range(4):
            chunked(wv[i], WvE[i * 128:(i + 1) * 128, :], 2)
        nc.sync.dma_start(out=wv5, in_=WvE[512:520, :])
        for i in range(2):
            chunked(wp[i], WpT[i * 128:(i + 1) * 128, :], 2)
        for i in range(GH):
            chunked(bmq[i][0:64, :], BMQ[i], 2)
            chunked(bmq[i][64:128, :], BMQ[i], 2)
        nc.sync.dma_start(out=et[0:64, :], in_=ET[:, :])
        nc.sync.dma_start(out=et[64:128, :], in_=ET[:, :])
        nc.sync.dma_start(out=tri, in_=TRI[:, :])
        nc.sync.dma_start(out=cmask, in_=CMASK[:, :])
        nc.sync.dma_start(out=one8j, in_=ONE8J[:, :])
        nc.sync.dma_start(out=one8b[64:72, :], in_=ONE8J[:, :])
        for t in range(2):
            nc.sync.dma_start(out=bq_t[:, t:t + 1], in_=BQ[t])
            nc.sync.dma_start(out=bk_t[:, t:t + 1], in_=BK[t])

        # ---- Q^T / K^T projections: out (d x i), contraction over c
        for wt, bt, out_t in ((wq, bq_t, qt), (wk, bk_t, kt)):
            for dt in range(2):
                for ib in range(2):
                    mmp = pmm.tile([128, 512], F32, tag="mm", name="mmp")
                    for ct in range(4):
                        nc.tensor.matmul(
                            mmp,
                            wt[ct][:, dt * 128:(dt + 1) * 128],
                            xt[ct][:, ib * 512:(ib + 1) * 512],
                            start=(ct == 0), stop=(ct == 3),
                        )
                    nc.vector.tensor_scalar_add(
                        out_t[dt][:, ib * 512:(ib + 1) * 512], mmp, bt[:, dt:dt + 1]
                    )

        # ---- V_ext: out (j x 260) per 128-token tile; bias+ones via extra row
        for jt in range(8):
            vp = pmm.tile([128, 260], F32, tag="mm", name="vp")
            for ct in range(4):
                nc.tensor.matmul(
                    vp,
                    xt[ct][:, jt * 128:(jt + 1) * 128],
                    wv[ct],
                    start=(ct == 0), stop=False,
                )
            nc.tensor.matmul(vp, one8j, wv5, start=False, stop=True)
            nc.vector.tensor_copy(ve[jt], vp)

        if stage <= 1:
            zs1 = szout.tile([128, 512], F32, tag="z", name="zs1")
            nc.vector.tensor_copy(zs1, qt[0][:, 0:512])
            nc.sync.dma_start(out=Z[0:128, :], in_=zs1)
            return

        # ---- attention
        for blk in range(2):
            q0 = blk * 512
            for hp in range(GH):
                dt, off = hp // 2, (hp % 2) * 64
                otp = pot.tile([65, 512], F32, tag="ot", name="otp")
                njt = 4 * (blk + 1)
                for jt in range(njt):
                    m = jt - 4 * blk          # >=0: diagonal-crossing tile
                    c0 = 128 * m if m >= 0 else 0
                    sp = ps.tile([128, 512], F32, tag="s", name="sp")
                    has_bias = blk == 0
                    nc.tensor.matmul(
                        sp[:, c0:],
                        kt[dt][off:off + 64, jt * 128:(jt + 1) * 128],
                        qt[dt][off:off + 64, q0 + c0:q0 + 512],
                        start=True, stop=not has_bias,
                    )
                    if has_bias:
                        nc.tensor.matmul(
                            sp[:, c0:],
                            et[off:off + 64, jt * 128:(jt + 1) * 128],
                            bmq[hp][off:off + 64, c0:512],
                            start=False, stop=True,
                        )
                    pt = spt.tile([128, 512], F32R, tag="pt", name="pt")
                    nc.scalar.activation(
                        pt[:, c0:], sp[:, c0:], AFT.Exp, bias=cmask[:, 0:1]
                    )
                    if m >= 0:
                        nc.vector.tensor_mul(
                            pt[:, c0:c0 + 128], pt[:, c0:c0 + 128], tri
                        )
                    if stage <= 2:
                        if blk == 0 and hp == 0 and jt == 0:
                            zs2 = szout.tile([128, 512], F32, tag="z", name="zs2")
                            nc.vector.tensor_copy(zs2, pt)
                            nc.sync.dma_start(out=Z[0:128, :], in_=zs2)
                        continue
                    nc.tensor.matmul(
                        otp[:, c0:],
                        ve[jt][:, 65 * hp:65 * hp + 65],
                        pt[:, c0:],
                        start=(jt == 0), stop=(jt == njt - 1),
                    )
                if stage <= 2:
                    continue
                den = sden.tile([72, 512], F32R, tag="den", name="den")
                nc.sync.dma_start(out=den[65:72, :], in_=ZERO7[:, :])
                with nc.allow_low_precision(reason="softmax denominator"):
                    nc.vector.reciprocal(den[64:65, :], otp[64:65, :])
                bcp = pbc.tile([64, 512], F32, tag="bc", name="bcp")
                nc.tensor.matmul(
                    bcp, one8b[64:72, 0:64], den[64:72, :],
                    start=True, stop=True,
                )
                bcs = sden.tile([64, 512], F32R, tag="bcs", name="bcs")
                nc.vector.tensor_copy(bcs, bcp)
                nc.vector.tensor_mul(
                    yt[dt][off:off + 64, q0:q0 + 512], otp[0:64, :], bcs
                )

        if stage == 2:
            return
        if stage == 3:
            zs3 = szout.tile([128, 512], F32, tag="z", name="zs3")
            nc.vector.tensor_copy(zs3, yt[0][:, 0:512])
            nc.sync.dma_start(out=Z[0:128, :], in_=zs3)
            return

        # ---- partial projection Z = y^T.T @ WpT_g
        for it in range(8):
            zp = pmm.tile([128, 512], F32, tag="mm", name="zp")
            for ct in range(2):
                nc.tensor.matmul(
                    zp,
                    yt[ct][:, it * 128:(it + 1) * 128],
                    wp[ct],
                    start=(ct == 0), stop=(ct == 1),
                )
            zs = szout.tile([128, 512], F32, tag="z", name="zs")
            nc.vector.tensor_copy(zs, zp)
            nc.sync.dma_start(out=Z[it * 128:(it + 1) * 128, :], in_=zs)


def _build(stage=99):
    nc = bacc.Bacc("TRN2", target_bir_lowering=False, debug=False,
                   num_devices=N_CORES)
    io = {}

    def din(name, shape, dt=F32R):
        io[name] = nc.dram_tensor(name, shape, dt, kind="ExternalInput").ap()

    din("xT", (C, T))
    din("WqT", (C, 256))
    din("WkT", (C, 256))
    din("WvE", (520, 260))
    din("WpT", (256, C))
    din("BQ", (2, 128, 1), F32)
    din("BK", (2, 128, 1), F32)
    din("BMQ", (GH, 64, T))
    din("ET", (64, T))
    din("TRI", (128, 128))
    din("CMASK", (128, 1), F32)
    din("ONE8J", (8, 128))
    din("ZERO7", (7, 512))
    io["Z"] = nc.dram_tensor("Z", (T, C), F32, kind="ExternalOutput").ap()

    with tile.TileContext(nc) as tc:
        _kernel_body(tc, io, stage)
    nc.compile()
    return nc


def _one8j():
    a = np.zeros((8, 128), np.float32)
    a[0, :] = 1.0
    return a


def _host_prep(x, attn_bias, Wq, bq, Wk, bk, Wv, bv, Wp, bp):
    """Build the 8 per-core input maps."""
    f = np.float32
    ET = np.zeros((64, T), f)
    for gj in range(64):
        ET[gj, gj * 8:(gj + 1) * 8] = 1.0
    TRI = (np.arange(128)[None, :] >= np.arange(128)[:, None]).astype(f)
    CMASK = np.zeros((128, 1), f)
    CMASK[15::16] = -1e30

    in_maps = []
    for core in range(N_CORES):
        b, g = core // 2, core % 2
        gs = slice(256 * g, 256 * (g + 1))
        WqT = np.ascontiguousarray((Wq[gs, :] * SCALE).T, dtype=f)
        WkT = np.ascontiguousarray(Wk[gs, :].T, dtype=f)
        WvE = np.zeros((520, 260), f)
        for hp in range(GH):
            r = slice(256 * g + 64 * hp, 256 * g + 64 * hp + 64)
            WvE[:C, 65 * hp:65 * hp + 64] = Wv[r, :].T
            WvE[C, 65 * hp:65 * hp + 64] = bv[r]
            WvE[C, 65 * hp + 64] = 1.0
        WpT = np.ascontiguousarray(Wp[:, gs].T, dtype=f)
        BMQ = np.zeros((GH, 64, T), f)
        for hp in range(GH):
            h = GH * g + hp
            BMQ[hp, :, :512] = np.repeat(attn_bias[h], 8, axis=0).T
        in_maps.append({
            "ONE8J": _one8j(),
            "ZERO7": np.zeros((7, 512), f),
            "xT": np.ascontiguousarray(x[b].T, dtype=f),
            "WqT": WqT,
            "WkT": WkT,
            "WvE": WvE,
            "WpT": WpT,
            "BQ": np.ascontiguousarray((bq[gs] * SCALE).reshape(2, 128, 1), f),
            "BK": np.ascontiguousarray(bk[gs].reshape(2, 128, 1), f),
            "BMQ": BMQ,
            "ET": ET,
            "TRI": TRI,
            "CMASK": CMASK,
        })
    return in_maps


def kernel(**inputs):
    global LAST_RESULTS
    if "nc" not in _CACHE:
        _CACHE["nc"] = _build()
    nc = _CACHE["nc"]

    in_maps = _host_prep(**{k: np.asarray(v) for k, v in inputs.items()})
    res = run_bass_kernel_spmd(nc, in_maps, core_ids=list(range(N_CORES)))
    LAST_RESULTS = res

    bp = np.asarray(inputs["bp"], np.float32)
    out = np.empty((B, T, C), np.float32)
    for b in range(B):
        out[b] = (np.asarray(res.results[2 * b]["Z"])
                  + np.asarray(res.results[2 * b + 1]["Z"])
                  + bp[None, :])
    return out



# revision 8
# speedup vs baseline: 1.9133x; 1.9133x over previous
"""Trainium2 Bass kernel for CausalSelfAttention with kron-structured bias and
column-masked causal attention.

Shapes (hardcoded): x (4,1024,512), H=8 heads, HD=64, attn_bias (8,64,64)
expanded by kron(ones(8,8)) onto the top-left 512x512 of the (1024,1024)
score matrix. Causal tril mask with every 16th column (j%16==15) zeroed.

Sharding: 8 cores = 4 batches x 2 head-groups (4 heads each). Every core runs
an identical program on its own slice.

v2 design (vs fp32r baseline):
  - all matmul operands bf16 (halves DMA bytes, removes the fp32r 4x penalty
    on <256-col matmuls, 2x DVE throughput on elementwise ops),
  - inputs packed into a few large HBM tensors, issued across BOTH hwdge
    queues (sync + scalar) so the ~600ns/issue serialization disappears,
  - the kron bias is folded into the score matmul: per-head K^T and Q^T are
    packed with ET (one-hot block expansion) / BMQ (repeated bias) in
    partitions 64..127, so one 128-deep matmul computes K.Q + ET.BMQ; the
    bias vanishes outside the 512x512 region because ET is zero for keys>=512
    and BMQ is zero for queries>=512,
  - V bias+ones column added by a fused vector op instead of a matmul,
  - softmax reciprocal via vector reciprocal_approx_fast ([1,512], ~5x
    faster than the 3.35us InstReciprocal),
  - Z partials written bf16 and summed on host.
"""

import sys
import types

import numpy as np
import ml_dtypes

import concourse.bass as bass
import concourse.bacc as bacc
import concourse.tile as tile
from concourse import mybir
from concourse.bass_utils import run_bass_kernel_spmd


def _ensure_axon_hooks():
    """bass_utils' trace path imports antenv.axon_hooks unconditionally; some
    images lack that module. Provide it (and register the real NTFF hook when
    the axon boot shim is available) so tracing degrades gracefully."""
    try:
        import antenv.axon_hooks  # noqa: F401
        return
    except ImportError:
        pass
    m = types.ModuleType("antenv.axon_hooks")
    m._hook = None
    m.set_axon_ntff_profile_hook = lambda h: setattr(m, "_hook", h)
    m.get_axon_ntff_profile_hook = lambda: m._hook
    sys.modules["antenv.axon_hooks"] = m
    try:
        import antenv
        antenv.axon_hooks = m
    except ImportError:
        pass
    try:
        from trn_agent_boot.trn_boot import _ntff_profile_via_ctypes
        m.set_axon_ntff_profile_hook(
            _ntff_profile_via_ctypes("/opt/axon/libaxon_pjrt.so")
        )
    except Exception:
        pass


_ensure_axon_hooks()

F32 = mybir.dt.float32
F32R = mybir.dt.float32r
BF16 = mybir.dt.bfloat16
AFT = mybir.ActivationFunctionType
ALU = mybir.AluOpType
BF = ml_dtypes.bfloat16

B, T, C, H = 4, 1024, 512, 8
HD = 64
SCALE = 1.0 / 8.0
GH = 4          # heads per core
N_CORES = 8

_CACHE = {}
LAST_RESULTS = None


def _kernel_body(tc, io, stage=99):
    nc = tc.nc

    from contextlib import ExitStack
    with ExitStack() as ctx:
        const = ctx.enter_context(tc.tile_pool(name="const", bufs=1))
        pmm = ctx.enter_context(tc.tile_pool(name="pmm", bufs=2, space="PSUM"))
        ps = ctx.enter_context(tc.tile_pool(name="ps", bufs=2, space="PSUM"))
        pot = ctx.enter_context(tc.tile_pool(name="pot", bufs=3, space="PSUM"))
        pbc = ctx.enter_context(tc.tile_pool(name="pbc", bufs=1, space="PSUM"))
        spt = ctx.enter_context(tc.tile_pool(name="spt", bufs=4))
        sden = ctx.enter_context(tc.tile_pool(name="sden", bufs=3))
        szout = ctx.enter_context(tc.tile_pool(name="szout", bufs=2))

        def ctile(shape, tag, dt=BF16):
            return const.tile(shape, dt, tag=tag, name=tag)

        xt = [ctile([128, T], f"xt{i}") for i in range(4)]
        wqk = ctile([128, 2048], "wqk")
        vw = ctile([128, 1040], "vw")
        vb = ctile([128, 260], "vb")
        keall = ctile([128, 4096], "keall")
        qball = ctile([128, 4096], "qball")
        wpt = ctile([128, 1024], "wpt")
        tri = ctile([128, 128], "tri")
        cmask = ctile([128, 1], "cmask", F32)
        bqbk = ctile([128, 4], "bqbk", F32)
        ones1 = ctile([1, 64], "ones1", BF16)
        ve = [ctile([128, 260], f"ve{i}") for i in range(8)]
        yt = [ctile([128, T], f"yt{i}") for i in range(2)]

        # ---- input DMAs: split across the two hwdge issue queues (sync +
        # scalar) so issue serialization (~600ns each) stays off the critical
        # path; critical tensors (bqbk, wqk, x) first on sync.
        nc.sync.dma_start(out=bqbk, in_=io["BQBK"][:, :])
        nc.sync.dma_start(out=wqk, in_=io["WQK"][:, :])
        for i in range(4):
            nc.sync.dma_start(out=xt[i], in_=io["XT"][i])
        nc.scalar.dma_start(out=cmask, in_=io["CMASK"][:, :])
        nc.scalar.dma_start(out=ones1, in_=io["ONES1"][:, :])
        nc.scalar.dma_start(out=tri, in_=io["TRI"][:, :])
        nc.scalar.dma_start(out=vb, in_=io["VB"][:, :])
        nc.scalar.dma_start(out=vw, in_=io["VW"][:, :])
        nc.scalar.dma_start(out=keall[64:128, :], in_=io["KER"][:, :])
        nc.scalar.dma_start(out=qball[64:128, :], in_=io["QBR"][:, :])
        nc.scalar.dma_start(out=wpt, in_=io["WPT"][:, :])

        # ---- Q/K projections into per-head packed tiles (rows 0..63).
        # proj 0 = Q (-> qball, bias cols 0..1), proj 1 = K (-> keall, 2..3)
        for proj, dest in ((0, qball), (1, keall)):
            for dt in range(2):
                for ib in range(2):
                    mmp = pmm.tile([128, 512], F32, tag="mm", name="mmp")
                    for ct in range(4):
                        nc.tensor.matmul(
                            mmp,
                            wqk[:, ct * 512 + proj * 256 + dt * 128:
                                ct * 512 + proj * 256 + (dt + 1) * 128],
                            xt[ct][:, ib * 512:(ib + 1) * 512],
                            start=(ct == 0), stop=(ct == 3),
                        )
                    for hh in range(2):
                        hp = 2 * dt + hh
                        nc.vector.tensor_scalar_add(
                            dest[0:64, hp * 1024 + ib * 512:
                                 hp * 1024 + (ib + 1) * 512],
                            mmp[hh * 64:(hh + 1) * 64, :],
                            bqbk[hh * 64:(hh + 1) * 64,
                                 proj * 2 + dt:proj * 2 + dt + 1],
                        )

        # ---- V_ext: out (j x 260) per 128-token tile; bias+ones via fused
        # vector add of the broadcast vb tile
        for jt in range(8):
            vp = pmm.tile([128, 260], F32, tag="mm", name="vp")
            for ct in range(4):
                nc.tensor.matmul(
                    vp,
                    xt[ct][:, jt * 128:(jt + 1) * 128],
                    vw[:, ct * 260:(ct + 1) * 260],
                    start=(ct == 0), stop=(ct == 3),
                )
            nc.vector.scalar_tensor_tensor(
                ve[jt], vp, 1.0, vb, op0=ALU.mult, op1=ALU.add
            )

        if stage <= 1:
            zs1 = szout.tile([128, 512], BF16, tag="z", name="zs1")
            nc.vector.tensor_copy(zs1, qball[:, 1024:1536])
            nc.sync.dma_start(out=io["Z"][:, 0:512], in_=zs1)
            return

        # ---- attention: per (head, query-block); kron bias folded into the
        # 128-deep score matmul via the ET/BMQ rows
        for hp in range(GH):
            h0 = hp * 1024
            for blk in range(2):
                q0 = blk * 512
                otp = pot.tile([65, 512], F32, tag="ot", name="otp")
                njt = 4 * (blk + 1)
                for jt in range(njt):
                    m = jt - 4 * blk      # >=0: diagonal-crossing tile
                    c0 = 128 * m if m >= 0 else 0
                    sp = ps.tile([128, 512], F32, tag="s", name="sp")
                    nc.tensor.matmul(
                        sp[:, c0:],
                        keall[:, h0 + jt * 128:h0 + (jt + 1) * 128],
                        qball[:, h0 + q0 + c0:h0 + q0 + 512],
                        start=True, stop=True,
                    )
                    pt = spt.tile([128, 512], BF16, tag="pt", name="pt")
                    nc.scalar.activation(
                        pt[:, c0:], sp[:, c0:], AFT.Exp, bias=cmask[:, 0:1]
                    )
                    if m >= 0:
                        nc.vector.tensor_mul(
                            pt[:, c0:c0 + 128], pt[:, c0:c0 + 128], tri
                        )
                    if stage <= 2:
                        if blk == 0 and hp == 0 and jt == 0:
                            zs2 = szout.tile([128, 512], BF16, tag="z",
                                             name="zs2")
                            nc.vector.tensor_copy(zs2, pt)
                            nc.sync.dma_start(out=io["Z"][:, 0:512], in_=zs2)
                        continue
                    nc.tensor.matmul(
                        otp[:, c0:],
                        ve[jt][:, 65 * hp:65 * hp + 65],
                        pt[:, c0:],
                        start=(jt == 0), stop=(jt == njt - 1),
                    )
                if stage <= 2:
                    continue
                dent = sden.tile([1, 512], F32, tag="dent", name="dent")
                nc.vector.tensor_copy(dent, otp[64:65, :])
                rec = sden.tile([1, 512], F32, tag="rec", name="rec")
                with nc.allow_low_precision(reason="softmax denominator"):
                    nc.vector.reciprocal_approx_fast(rec, dent)
                recb = sden.tile([1, 512], BF16, tag="recb", name="recb")
                nc.vector.tensor_copy(recb, rec)
                bcp = pbc.tile([64, 512], F32, tag="bc", name="bcp")
                nc.tensor.matmul(
                    bcp, ones1, recb,
                    start=True, stop=True,
                )
                bcs = sden.tile([64, 512], BF16, tag="bcs", name="bcs")
                nc.vector.tensor_copy(bcs, bcp)
                nc.vector.tensor_mul(
                    yt[hp // 2][(hp % 2) * 64:(hp % 2) * 64 + 64,
                                q0:q0 + 512],
                    otp[0:64, :], bcs
                )

        if stage == 2:
            return
        if stage == 3:
            zs3 = szout.tile([128, 512], BF16, tag="z", name="zs3")
            nc.vector.tensor_copy(zs3, yt[0][:, 0:512])
            nc.sync.dma_start(out=io["Z"][:, 0:512], in_=zs3)
            return

        # ---- partial projection Z = y^T.T @ WpT_g (bf16 out, host sums)
        for it in range(8):
            zp = pmm.tile([128, 512], F32, tag="mm", name="zp")
            for ct in range(2):
                nc.tensor.matmul(
                    zp,
                    yt[ct][:, it * 128:(it + 1) * 128],
                    wpt[:, ct * 512:(ct + 1) * 512],
                    start=(ct == 0), stop=(ct == 1),
                )
            zs = szout.tile([128, 512], BF16, tag="z", name="zs")
            nc.vector.tensor_copy(zs, zp)
            nc.sync.dma_start(
                out=io["Z"][:, it * 512:(it + 1) * 512], in_=zs
            )


def _build(stage=99):
    nc = bacc.Bacc("TRN2", target_bir_lowering=False, debug=False,
                   num_devices=N_CORES)
    io = {}

    def din(name, shape, dt=BF16):
        io[name] = nc.dram_tensor(name, shape, dt, kind="ExternalInput").ap()

    din("XT", (4, 128, T))
    din("WQK", (128, 2048))
    din("VW", (128, 1040))
    din("VB", (128, 260))
    din("KER", (64, 4096))
    din("QBR", (64, 4096))
    din("WPT", (128, 1024))
    din("TRI", (128, 128))
    din("CMASK", (128, 1), F32)
    din("BQBK", (128, 4), F32)
    din("ONES1", (1, 64))
    io["Z"] = nc.dram_tensor("Z", (128, 4096), BF16, kind="ExternalOutput").ap()

    with tile.TileContext(nc) as tc:
        _kernel_body(tc, io, stage)
    nc.compile()
    return nc


def _host_prep(x, attn_bias, Wq, bq, Wk, bk, Wv, bv, Wp, bp):
    """Build the 8 per-core input maps."""
    f = np.float32

    # ET pattern (key-block one-hot) / TRI / CMASK are core-independent
    KER = np.zeros((64, 4096), f)
    for hp in range(GH):
        for gj in range(64):
            KER[gj, hp * 1024 + gj * 8:hp * 1024 + (gj + 1) * 8] = 1.0
    TRI = (np.arange(128)[None, :] >= np.arange(128)[:, None]).astype(f)
    CMASK = np.zeros((128, 1), f)
    CMASK[15::16] = -1e30
    ONES1 = np.ones((1, 64), f)

    in_maps = []
    for core in range(N_CORES):
        b, g = core // 2, core % 2
        gs = slice(256 * g, 256 * (g + 1))

        XT = np.ascontiguousarray(
            x[b].T.reshape(4, 128, T), dtype=f).astype(BF)

        WQK = np.zeros((128, 2048), f)
        WqT = (Wq[gs, :] * SCALE).T      # (512, 256)
        WkT = Wk[gs, :].T                # (512, 256)
        for ct in range(4):
            rs = slice(128 * ct, 128 * (ct + 1))
            WQK[:, ct * 512:ct * 512 + 256] = WqT[rs, :]
            WQK[:, ct * 512 + 256:(ct + 1) * 512] = WkT[rs, :]

        VW = np.zeros((128, 1040), f)
        VB = np.zeros((128, 260), f)
        for hp in range(GH):
            r = slice(256 * g + 64 * hp, 256 * g + 64 * hp + 64)
            WvT = Wv[r, :].T             # (512, 64)
            for ct in range(4):
                VW[:, ct * 260 + 65 * hp:ct * 260 + 65 * hp + 64] = \
                    WvT[128 * ct:128 * (ct + 1), :]
            VB[:, 65 * hp:65 * hp + 64] = bv[r][None, :]
            VB[:, 65 * hp + 64] = 1.0

        QBR = np.zeros((64, 4096), f)
        for hp in range(GH):
            h = GH * g + hp
            QBR[:, hp * 1024:hp * 1024 + 512] = \
                np.repeat(attn_bias[h], 8, axis=0).T

        WPT = np.zeros((128, 1024), f)
        for ct in range(2):
            r = slice(256 * g + 128 * ct, 256 * g + 128 * (ct + 1))
            WPT[:, ct * 512:(ct + 1) * 512] = Wp[:, r].T

        BQBK = np.zeros((128, 4), f)
        BQBK[:, 0] = (bq[gs] * SCALE)[0:128]
        BQBK[:, 1] = (bq[gs] * SCALE)[128:256]
        BQBK[:, 2] = bk[gs][0:128]
        BQBK[:, 3] = bk[gs][128:256]

        in_maps.append({
            "XT": XT,
            "WQK": WQK.astype(BF),
            "VW": VW.astype(BF),
            "VB": VB.astype(BF),
            "KER": KER.astype(BF),
            "QBR": QBR.astype(BF),
            "WPT": WPT.astype(BF),
            "TRI": TRI.astype(BF),
            "CMASK": CMASK,
            "BQBK": BQBK,
            "ONES1": ONES1.astype(BF),
        })
    return in_maps


def kernel(**inputs):
    global LAST_RESULTS
    if "nc" not in _CACHE:
        _CACHE["nc"] = _build()
    nc = _CACHE["nc"]

    in_maps = _host_prep(**{k: np.asarray(v) for k, v in inputs.items()})
    res = run_bass_kernel_spmd(nc, in_maps, core_ids=list(range(N_CORES)))
    LAST_RESULTS = res

    bp = np.asarray(inputs["bp"], np.float32)
    out = np.empty((B, T, C), np.float32)
    for b in range(B):
        z0 = np.asarray(res.results[2 * b]["Z"], np.float32)
        z1 = np.asarray(res.results[2 * b + 1]["Z"], np.float32)
        z = (z0 + z1).reshape(128, 8, 512).transpose(1, 0, 2).reshape(T, C)
        out[b] = z + bp[None, :]
    return out
